# revision 2
# baseline (speedup 1.0000x reference)
"""Trainium2 Bass kernel for CRF Viterbi decode (nn_CRF).

Problem (hardcoded): x[64, 512, 1024] @ kernel[1024, 128] + bias -> logits
[B, T, U]; boundary energies added on first/last timestep; Viterbi decode
with transition matrix chain_kernel[128, 128]; returns tags as float32.

Strategy
--------
Data-parallel over 8 NeuronCores: 8 batch elements per core, split into
2 pipeline groups of 4 so the serial per-timestep recurrence of one group
overlaps the other group's engine work.

Device (per core):
  1. logits matmul: x^T (pre-transposed on host to [D, (t, b)] layout)
     against kernel tiles, accumulated in PSUM over K=1024, evacuated by
     the scalar engine (bias fused) into SBUF-resident logitsT[u, (t,b)].
  2. Viterbi forward scan (511 serial steps). Per step and group:
       - PE rebuilds the group's PSUM score tile with ONE float32r real
         matmul (lhsT=trans, rhs=identity tiled along batches with
         stride-0): sc[j, (b,i)] = trans[i, j]. float32r streams 1
         cycle/column (vs 2 for fp32 transposes), halving PE cost; the
         tf32-style input rounding it implies is applied to trans ON THE
         HOST first, so device arithmetic stays bit-reproducible.
       - PE accumulates 4 broadcast transposes of v_{t-1} columns
         (exact fp32): sc[j, (b,i)] += v[b, i].
       - DVE does the segmented reduce-max in TWO halves (2 batches
         each) so the logit-add and next step's broadcasts start earlier
         (shortens the recurrence chain below the DVE-pole).
       - GPSIMD adds the logit columns: vh = mx + logitsT cols. Putting
         this tensor_tensor on GPSIMD keeps the DVE queue pure reduces
         (a DVE-resident add gets scheduled after the OTHER group's
         reduce, adding ~650ns to the chain).
     Only max VALUES are kept (no argmax on device) - backpointers are
     reconstructed exactly on the host from vhist, since fp32 add/max
     here are bit-exact reproducible.
  3. vhist is DMAd out chunk-by-chunk as the scan progresses.

Host: round trans to tf32 (matching the device float32r path), shard and
pre-transpose inputs, run SPMD on cores 0-7, then backtrace:
  tag_t = argmax_i(v_t[b, i] + trans_r[i, tag_{t+1}]) - bit-identical to
the backpointers the device forward pass implies.
"""

import os

import numpy as np

import concourse.bass as bass
import concourse.mybir as mybir
from concourse.tile import TileContext
from concourse.bass_utils import run_bass_kernel_spmd

F32 = mybir.dt.float32
F32R = mybir.dt.float32r

# Problem constants
B, T, D, U = 64, 512, 1024, 128
NCORES = 8
BL = B // NCORES           # batches per core (8)
NG = 2                     # batch pipeline groups
GB = BL // NG              # batches per group (4)

last_results = None        # BassKernelResults of the most recent kernel() run


def round_tf32(x):
    """Round-to-nearest-even fp32 -> tf32-style (10 mantissa bits), the
    rounding TRN2 applies to float32r matmul operands (verified on HW:
    a DVE fp32->f32r copy is idempotent on these values)."""
    u = np.ascontiguousarray(x, dtype=np.float32).view(np.uint32)
    r = ((u.astype(np.uint64) + 0xFFF + ((u >> 13) & 1))
         & 0xFFFFE000).astype(np.uint32)
    return r.view(np.float32)


def split_multi_waits(nc):
    """The walrus build in this container encodes at most ONE sync wait per
    compute/DMA instruction ("Too many sync wait commands" otherwise). Hoist
    all but the last wait of any multi-wait instruction onto standalone
    same-engine EventSemaphore ops placed immediately before it (engine
    queues execute in order, so semantics are preserved)."""
    for f in nc.m.functions:
        for blk in f.blocks:
            new_insts = []
            changed = False
            for inst in blk.instructions:
                si = inst.sync_info
                if si is not None and len(si.on_wait) > 1:
                    waits = list(si.on_wait)
                    for k, w in enumerate(waits[:-1]):
                        new_insts.append(mybir.InstEventSemaphore(
                            name=f"{inst.name}-sw{k}",
                            engine=inst.engine,
                            ins=[], outs=[],
                            sync_info=mybir.SyncInfo(on_wait=[w], on_update=[]),
                        ))
                    inst.sync_info = mybir.SyncInfo(
                        on_wait=[waits[-1]], on_update=list(si.on_update))
                    changed = True
                new_insts.append(inst)
            if changed:
                blk.instructions = new_insts
    return nc


def build_program(t_steps=T, d_dim=D, split_waits=True, loop_reps=None):
    nt = t_steps * BL                       # columns in (t, b) layout
    ch = min(512, nt)                       # DMA/matmul chunk width
    nch = nt // ch
    kblocks = d_dim // 128

    nc = bass.Bass(trn_type="TRN2")

    xdt = nc.dram_tensor("xdt", [d_dim, nt], F32, kind="ExternalInput")
    ker = nc.dram_tensor("ker", [d_dim, U], F32, kind="ExternalInput")
    translhs = nc.dram_tensor("translhs", [U, U], F32, kind="ExternalInput")
    ident = nc.dram_tensor("ident", [U, U], F32, kind="ExternalInput")
    lbv = nc.dram_tensor("lbv", [U, 1], F32, kind="ExternalInput")
    rbv = nc.dram_tensor("rbv", [U, 1], F32, kind="ExternalInput")
    biasrow = nc.dram_tensor("biasrow", [1, U], F32, kind="ExternalInput")
    onesrow = nc.dram_tensor("onesrow", [1, 512], F32, kind="ExternalInput")
    vout = nc.dram_tensor("vout", [U, nt], F32, kind="ExternalOutput")

    with TileContext(nc) as tc:
        with (
            tc.tile_pool(name="const", bufs=1) as cpool,
            tc.tile_pool(name="xp", bufs=10) as xpool,
            tc.tile_pool(name="big", bufs=1) as bigpool,
            tc.tile_pool(name="mx", bufs=3) as mxpool,
            tc.tile_pool(name="mmps", bufs=1, space="PSUM") as mmpool,
            tc.tile_pool(name="scps0", bufs=2, space="PSUM") as scpool0,
            tc.tile_pool(name="scps1", bufs=2, space="PSUM") as scpool1,
        ):
            # ---- constants into SBUF ----
            ker_sb = []
            for kb in range(kblocks):
                kt = cpool.tile([128, U], F32, tag=f"ker{kb}")
                nc.sync.dma_start(out=kt[:, :], in_=ker[kb * 128:(kb + 1) * 128, :])
                ker_sb.append(kt)
            trans_sb = cpool.tile([U, U], F32, tag="trans")
            nc.sync.dma_start(out=trans_sb[:, :], in_=translhs[:, :])
            # float32r view of trans for the 1-cycle/col refresh matmul;
            # the host pre-rounds trans so this copy is value-preserving.
            trans_r = cpool.tile([U, U], F32R, tag="transr")
            nc.vector.tensor_copy(out=trans_r[:, :], in_=trans_sb[:, :])
            ident_sb = cpool.tile([U, U], F32, tag="ident")
            nc.sync.dma_start(out=ident_sb[:, :], in_=ident[:, :])
            ident_r = cpool.tile([U, U], F32R, tag="identr")
            nc.vector.tensor_copy(out=ident_r[:, :], in_=ident_sb[:, :])
            lb_sb = cpool.tile([U, 1], F32, tag="lb")
            nc.sync.dma_start(out=lb_sb[:, :], in_=lbv[:, :])
            rb_sb = cpool.tile([U, 1], F32, tag="rb")
            nc.sync.dma_start(out=rb_sb[:, :], in_=rbv[:, :])
            biasrow_sb = cpool.tile([1, U], F32, tag="biasrow")
            nc.sync.dma_start(out=biasrow_sb[:, :], in_=biasrow[:, :])
            onesrow_sb = cpool.tile([1, 512], F32, tag="onesrow")
            nc.sync.dma_start(out=onesrow_sb[:, :], in_=onesrow[:, :])

            logitsT = bigpool.tile([U, nt], F32, tag="logitsT")
            # per-group v history; group g columns: t * GB + bb
            vh = [bigpool.tile([U, nt // NG], F32, tag=f"vh{g}",
                                name=f"vh{g}")
                  for g in range(NG)]

            # ---- phase 1: logits = kernel.T @ x (+bias) ----
            for c in range(nch):
                ps = mmpool.tile([128, ch], F32, tag="mm")
                for kb in range(kblocks):
                    xt = xpool.tile([128, ch], F32, tag="x")
                    nc.sync.dma_start(
                        out=xt[:, :],
                        in_=xdt[kb * 128:(kb + 1) * 128, c * ch:(c + 1) * ch],
                    )
                    nc.tensor.matmul(
                        out=ps[:, :], lhsT=ker_sb[kb][:, :], rhs=xt[:, :],
                        start=(kb == 0), stop=False,
                    )
                nc.tensor.matmul(
                    out=ps[:, :], lhsT=biasrow_sb[0:1, :],
                    rhs=onesrow_sb[0:1, 0:ch], start=False, stop=True,
                )
                nc.scalar.copy(
                    out=logitsT[:, c * ch:(c + 1) * ch], in_=ps[:, :],
                )

            # right boundary folded into the last timestep's logits
            nc.vector.tensor_scalar_add(
                out=logitsT[:, (t_steps - 1) * BL:],
                in0=logitsT[:, (t_steps - 1) * BL:],
                scalar1=rb_sb[:, 0:1],
            )

            # ---- phase 2: Viterbi forward scan ----
            steps_per_chunk = ch // BL
            gch = steps_per_chunk * GB          # per-group chunk width
            import contextlib
            rep_ctx = (tc.For_i(0, loop_reps, 1) if loop_reps
                       else contextlib.nullcontext())
            with rep_ctx:
             # v_0 = logits_0 + left boundary
             for g in range(NG):
                nc.vector.tensor_scalar_add(
                    out=vh[g][:, 0:GB], in0=logitsT[:, g * GB:(g + 1) * GB],
                    scalar1=lb_sb[:, 0:1],
                )
             for t in range(1, t_steps):
                 for g in range(NG):
                     lcols0 = t * BL + g * GB    # logitsT columns
                     vcols0 = t * GB             # vh[g] columns
                     pcol0 = (t - 1) * GB
                     sc = (scpool0 if g == 0 else scpool1).tile(
                         [128, GB * U], F32, tag=f"sc{g}")
                     # refresh sc with trans^T tiled over batches: ONE
                     # float32r real matmul, out[j,(b,i)] = trans[i,j]
                     nc.tensor.matmul(
                         out=sc[:, :],
                         lhsT=trans_r[:, :],
                         rhs=ident_r[:, :].rearrange(
                             "p (x i) -> p x i", x=1
                         ).broadcast_to([U, GB, U]),
                         start=True, stop=False,
                         skip_group_check=True,
                     )
                     # v_{t-1} broadcasts accumulate on top (exact fp32)
                     for bb in range(GB):
                         vcol = vh[g][:, pcol0 + bb:pcol0 + bb + 1]
                         nc.tensor.matmul(
                             out=sc[:, bb * U:(bb + 1) * U],
                             lhsT=vcol.broadcast_to([U, U]),
                             rhs=ident_sb[:, :],
                             start=False, stop=(bb == GB - 1),
                             skip_group_check=True, is_transpose=True,
                         )
                     # segmented reduce-max in two halves; GPSIMD adds the
                     # logit columns so DVE stays a pure reduce queue
                     mx = mxpool.tile([U, GB], F32, tag=f"mx{g}")
                     hw_ = GB // 2
                     for h in range(2):
                         nc.vector.tensor_reduce(
                             out=mx[:, h * hw_:(h + 1) * hw_],
                             in_=sc[:, h * hw_ * U:(h + 1) * hw_ * U]
                                 .rearrange("p (b i) -> p b i", i=U),
                             axis=mybir.AxisListType.X,
                             op=mybir.AluOpType.max,
                         )
                         nc.gpsimd.tensor_tensor(
                             out=vh[g][:, vcols0 + h * hw_:
                                       vcols0 + (h + 1) * hw_],
                             in0=mx[:, h * hw_:(h + 1) * hw_],
                             in1=logitsT[:, lcols0 + h * hw_:
                                         lcols0 + (h + 1) * hw_],
                             op=mybir.AluOpType.add,
                         )
                 if (t + 1) % steps_per_chunk == 0:
                     c = (t + 1) // steps_per_chunk - 1
                     for g in range(NG):
                         nc.sync.dma_start(
                             out=vout[:, g * (nt // NG) + c * gch:
                                      g * (nt // NG) + (c + 1) * gch],
                             in_=vh[g][:, c * gch:(c + 1) * gch],
                         )
    return split_multi_waits(nc) if split_waits else nc


def make_in_map(x_core, ker, bias, trans, lb, rb, t_steps=T, d_dim=D):
    """x_core: [BL, t_steps, d_dim] float32. NOTE: pass tf32-pre-rounded
    trans (round_tf32) for bit-consistency with the device float32r path."""
    nt = t_steps * BL
    xdt = np.ascontiguousarray(x_core.transpose(2, 1, 0)).reshape(d_dim, nt)
    return {
        "xdt": xdt.astype(np.float32),
        "ker": np.ascontiguousarray(ker, dtype=np.float32),
        "biasrow": np.ascontiguousarray(bias, dtype=np.float32).reshape(1, U),
        "onesrow": np.ones((1, 512), dtype=np.float32),
        "translhs": np.ascontiguousarray(trans, dtype=np.float32),
        "ident": np.eye(U, dtype=np.float32),
        "lbv": np.ascontiguousarray(lb, dtype=np.float32).reshape(U, 1),
        "rbv": np.ascontiguousarray(rb, dtype=np.float32).reshape(U, 1),
    }


def backtrace(v, trans):
    """v: [b, t, u] forward max values; trans: [u, u] (tf32-rounded, as used
    on device). Returns int tags [b, t]."""
    nb, nt, nu = v.shape
    tags = np.zeros((nb, nt), dtype=np.int64)
    cur = np.argmax(v[:, -1, :], axis=1)
    tags[:, -1] = cur
    for t in range(nt - 2, -1, -1):
        scores = v[:, t, :] + trans[:, cur].T     # fp32, same as device order
        cur = np.argmax(scores, axis=1)
        tags[:, t] = cur
    return tags


def vout_to_v(vout_core, t_steps=T):
    """vout [U, (g, t, bb)] -> v [BL, t, U] with b = g * GB + bb."""
    v = vout_core.reshape(U, NG, t_steps, GB)     # [u, g, t, bb]
    return np.ascontiguousarray(v.transpose(1, 3, 2, 0).reshape(BL, t_steps, U))


def kernel(x, kernel, bias, chain_kernel, left_boundary, right_boundary):
    x = np.asarray(x, dtype=np.float32)
    ker = np.asarray(kernel, dtype=np.float32)
    bias = np.asarray(bias, dtype=np.float32)
    trans = round_tf32(chain_kernel)
    lb = np.asarray(left_boundary, dtype=np.float32)
    rb = np.asarray(right_boundary, dtype=np.float32)

    nc = build_program()
    in_maps = [
        make_in_map(x[c * BL:(c + 1) * BL], ker, bias, trans, lb, rb)
        for c in range(NCORES)
    ]
    kwargs = {}
    if os.environ.get("CRF_TRACE"):
        kwargs = {"trace": True, "tmpdir": os.environ.get("CRF_TRACE_DIR") or None}
    res = run_bass_kernel_spmd(nc, in_maps, core_ids=list(range(NCORES)), **kwargs)
    global last_results
    last_results = res
    v = np.concatenate(
        [vout_to_v(np.asarray(r["vout"])) for r in res.results], axis=0)
    tags = backtrace(v, trans)
    return tags.astype(np.float32)


# revision 4
# speedup vs baseline: 1.1555x; 1.1555x over previous
"""Trainium2 Bass kernel for CRF Viterbi decode (nn_CRF).

Problem (hardcoded): x[64, 512, 1024] @ kernel[1024, 128] + bias -> logits
[B, T, U]; boundary energies added on first/last timestep; Viterbi decode
with transition matrix chain_kernel[128, 128]; returns tags as float32.

Strategy
--------
Data-parallel over 8 NeuronCores: 8 batch elements per core, split into
2 pipeline groups of 4 so the serial per-timestep recurrence of one group
overlaps the other group's engine work.

Device (per core):
  1. logits matmul: x^T (pre-transposed on host to [D, (t, b)] layout)
     against kernel tiles, accumulated in PSUM over K=1024, evacuated by
     the scalar engine (bias fused) into SBUF-resident logitsT[u, (t,b)].
  2. Viterbi forward scan (511 serial steps). Per step and group:
       - PE rebuilds the group's PSUM score tile with 4 exact fp32
         transposes of trans against the identity (PSUM accumulation
         group start): sc[j, (b,i)] = trans[i, j].
       - PE accumulates 4 broadcast transposes of v_{t-1} columns
         (exact fp32): sc[j, (b,i)] += v[b, i].
       - DVE does the segmented reduce-max in TWO halves (2 batches
         each) so the logit-add and next step's broadcasts start earlier
         (shortens the recurrence chain below the DVE-pole).
       - GPSIMD adds the logit columns: vh = mx + logitsT cols. Putting
         this tensor_tensor on GPSIMD keeps the DVE queue pure reduces
         (a DVE-resident add gets scheduled after the OTHER group's
         reduce, adding ~650ns to the chain).
     Only max VALUES are kept (no argmax on device) - backpointers are
     reconstructed exactly on the host from vhist, since fp32 add/max
     here are bit-exact reproducible.
  3. vhist is DMAd out chunk-by-chunk as the scan progresses.

Host: shard and pre-transpose inputs, run SPMD on cores 0-7, then
backtrace: tag_t = argmax_i(v_t[b, i] + trans[i, tag_{t+1}]) -
bit-identical to the backpointers the device forward pass implies.
"""

import os

import numpy as np

import concourse.bass as bass
import concourse.mybir as mybir
from concourse.tile import TileContext
from concourse.bass_utils import run_bass_kernel_spmd

F32 = mybir.dt.float32
F32R = mybir.dt.float32r

# Problem constants
B, T, D, U = 64, 512, 1024, 128
NCORES = 8
BL = B // NCORES           # batches per core (8)
NG = 2                     # batch pipeline groups
GB = BL // NG              # batches per group (4)

last_results = None        # BassKernelResults of the most recent kernel() run


def round_tf32(x):
    """Round-to-nearest-even fp32 -> tf32-style (10 mantissa bits), the
    rounding TRN2 applies to float32r matmul operands (verified on HW:
    a DVE fp32->f32r copy is idempotent on these values)."""
    u = np.ascontiguousarray(x, dtype=np.float32).view(np.uint32)
    r = ((u.astype(np.uint64) + 0xFFF + ((u >> 13) & 1))
         & 0xFFFFE000).astype(np.uint32)
    return r.view(np.float32)


def split_multi_waits(nc):
    """The walrus build in this container encodes at most ONE sync wait per
    compute/DMA instruction ("Too many sync wait commands" otherwise). Hoist
    all but the last wait of any multi-wait instruction onto standalone
    same-engine EventSemaphore ops placed immediately before it (engine
    queues execute in order, so semantics are preserved)."""
    for f in nc.m.functions:
        for blk in f.blocks:
            new_insts = []
            changed = False
            for inst in blk.instructions:
                si = inst.sync_info
                if si is not None and len(si.on_wait) > 1:
                    waits = list(si.on_wait)
                    for k, w in enumerate(waits[:-1]):
                        new_insts.append(mybir.InstEventSemaphore(
                            name=f"{inst.name}-sw{k}",
                            engine=inst.engine,
                            ins=[], outs=[],
                            sync_info=mybir.SyncInfo(on_wait=[w], on_update=[]),
                        ))
                    inst.sync_info = mybir.SyncInfo(
                        on_wait=[waits[-1]], on_update=list(si.on_update))
                    changed = True
                new_insts.append(inst)
            if changed:
                blk.instructions = new_insts
    return nc


def build_program(t_steps=T, d_dim=D, split_waits=True, loop_reps=None):
    nt = t_steps * BL                       # columns in (t, b) layout
    ch = min(512, nt)                       # DMA/matmul chunk width
    nch = nt // ch
    kblocks = d_dim // 128

    nc = bass.Bass(trn_type="TRN2")

    xdt = nc.dram_tensor("xdt", [d_dim, nt], F32, kind="ExternalInput")
    ker = nc.dram_tensor("ker", [d_dim, U], F32, kind="ExternalInput")
    translhs = nc.dram_tensor("translhs", [U, U], F32, kind="ExternalInput")
    ident = nc.dram_tensor("ident", [U, U], F32, kind="ExternalInput")
    lbv = nc.dram_tensor("lbv", [U, 1], F32, kind="ExternalInput")
    rbv = nc.dram_tensor("rbv", [U, 1], F32, kind="ExternalInput")
    biasrow = nc.dram_tensor("biasrow", [1, U], F32, kind="ExternalInput")
    onesrow = nc.dram_tensor("onesrow", [1, 512], F32, kind="ExternalInput")
    vout = nc.dram_tensor("vout", [U, nt], F32, kind="ExternalOutput")

    with TileContext(nc) as tc:
        with (
            tc.tile_pool(name="const", bufs=1) as cpool,
            tc.tile_pool(name="xp", bufs=10) as xpool,
            tc.tile_pool(name="big", bufs=1) as bigpool,
            tc.tile_pool(name="mx", bufs=3) as mxpool,
            tc.tile_pool(name="mmps", bufs=1, space="PSUM") as mmpool,
            tc.tile_pool(name="scps0", bufs=2, space="PSUM") as scpool0,
            tc.tile_pool(name="scps1", bufs=2, space="PSUM") as scpool1,
        ):
            # ---- constants into SBUF ----
            ker_sb = []
            for kb in range(kblocks):
                kt = cpool.tile([128, U], F32, tag=f"ker{kb}")
                nc.sync.dma_start(out=kt[:, :], in_=ker[kb * 128:(kb + 1) * 128, :])
                ker_sb.append(kt)
            trans_sb = cpool.tile([U, U], F32, tag="trans")
            nc.sync.dma_start(out=trans_sb[:, :], in_=translhs[:, :])
            ident_sb = cpool.tile([U, U], F32, tag="ident")
            nc.sync.dma_start(out=ident_sb[:, :], in_=ident[:, :])
            lb_sb = cpool.tile([U, 1], F32, tag="lb")
            nc.sync.dma_start(out=lb_sb[:, :], in_=lbv[:, :])
            rb_sb = cpool.tile([U, 1], F32, tag="rb")
            nc.sync.dma_start(out=rb_sb[:, :], in_=rbv[:, :])
            biasrow_sb = cpool.tile([1, U], F32, tag="biasrow")
            nc.sync.dma_start(out=biasrow_sb[:, :], in_=biasrow[:, :])
            onesrow_sb = cpool.tile([1, 512], F32, tag="onesrow")
            nc.sync.dma_start(out=onesrow_sb[:, :], in_=onesrow[:, :])

            logitsT = bigpool.tile([U, nt], F32, tag="logitsT")
            # per-group v history; group g columns: t * GB + bb
            vh = [bigpool.tile([U, nt // NG], F32, tag=f"vh{g}",
                                name=f"vh{g}")
                  for g in range(NG)]

            # ---- phase 1: logits = kernel.T @ x (+bias) ----
            for c in range(nch):
                ps = mmpool.tile([128, ch], F32, tag="mm")
                for kb in range(kblocks):
                    xt = xpool.tile([128, ch], F32, tag="x")
                    nc.sync.dma_start(
                        out=xt[:, :],
                        in_=xdt[kb * 128:(kb + 1) * 128, c * ch:(c + 1) * ch],
                    )
                    nc.tensor.matmul(
                        out=ps[:, :], lhsT=ker_sb[kb][:, :], rhs=xt[:, :],
                        start=(kb == 0), stop=False,
                    )
                nc.tensor.matmul(
                    out=ps[:, :], lhsT=biasrow_sb[0:1, :],
                    rhs=onesrow_sb[0:1, 0:ch], start=False, stop=True,
                )
                nc.scalar.copy(
                    out=logitsT[:, c * ch:(c + 1) * ch], in_=ps[:, :],
                )

            # right boundary folded into the last timestep's logits
            nc.vector.tensor_scalar_add(
                out=logitsT[:, (t_steps - 1) * BL:],
                in0=logitsT[:, (t_steps - 1) * BL:],
                scalar1=rb_sb[:, 0:1],
            )

            # ---- phase 2: Viterbi forward scan ----
            steps_per_chunk = ch // BL
            gch = steps_per_chunk * GB          # per-group chunk width
            import contextlib
            rep_ctx = (tc.For_i(0, loop_reps, 1) if loop_reps
                       else contextlib.nullcontext())
            with rep_ctx:
             # v_0 = logits_0 + left boundary
             for g in range(NG):
                nc.vector.tensor_scalar_add(
                    out=vh[g][:, 0:GB], in0=logitsT[:, g * GB:(g + 1) * GB],
                    scalar1=lb_sb[:, 0:1],
                )
             for t in range(1, t_steps):
                 for g in range(NG):
                     lcols0 = t * BL + g * GB    # logitsT columns
                     vcols0 = t * GB             # vh[g] columns
                     pcol0 = (t - 1) * GB
                     sc = (scpool0 if g == 0 else scpool1).tile(
                         [128, GB * U], F32, tag=f"sc{g}")
                     # refresh sc with trans^T per batch (exact fp32
                     # transposes): sc[j, (b,i)] = trans[i, j]
                     for bb in range(GB):
                         nc.tensor.matmul(
                             out=sc[:, bb * U:(bb + 1) * U],
                             lhsT=trans_sb[:, :], rhs=ident_sb[:, :],
                             start=(bb == 0), stop=False,
                             skip_group_check=True, is_transpose=True,
                         )
                     # v_{t-1} broadcasts accumulate on top (exact fp32)
                     for bb in range(GB):
                         vcol = vh[g][:, pcol0 + bb:pcol0 + bb + 1]
                         nc.tensor.matmul(
                             out=sc[:, bb * U:(bb + 1) * U],
                             lhsT=vcol.broadcast_to([U, U]),
                             rhs=ident_sb[:, :],
                             start=False, stop=(bb == GB - 1),
                             skip_group_check=True, is_transpose=True,
                         )
                     # segmented reduce-max in two halves; GPSIMD adds the
                     # logit columns so DVE stays a pure reduce queue
                     mx = mxpool.tile([U, GB], F32, tag=f"mx{g}")
                     hw_ = GB // 2
                     for h in range(2):
                         nc.vector.tensor_reduce(
                             out=mx[:, h * hw_:(h + 1) * hw_],
                             in_=sc[:, h * hw_ * U:(h + 1) * hw_ * U]
                                 .rearrange("p (b i) -> p b i", i=U),
                             axis=mybir.AxisListType.X,
                             op=mybir.AluOpType.max,
                         )
                         nc.gpsimd.tensor_tensor(
                             out=vh[g][:, vcols0 + h * hw_:
                                       vcols0 + (h + 1) * hw_],
                             in0=mx[:, h * hw_:(h + 1) * hw_],
                             in1=logitsT[:, lcols0 + h * hw_:
                                         lcols0 + (h + 1) * hw_],
                             op=mybir.AluOpType.add,
                         )
                 if (t + 1) % steps_per_chunk == 0:
                     c = (t + 1) // steps_per_chunk - 1
                     for g in range(NG):
                         nc.sync.dma_start(
                             out=vout[:, g * (nt // NG) + c * gch:
                                      g * (nt // NG) + (c + 1) * gch],
                             in_=vh[g][:, c * gch:(c + 1) * gch],
                         )
    return split_multi_waits(nc) if split_waits else nc


def make_in_map(x_core, ker, bias, trans, lb, rb, t_steps=T, d_dim=D):
    """x_core: [BL, t_steps, d_dim] float32."""
    nt = t_steps * BL
    xdt = np.ascontiguousarray(x_core.transpose(2, 1, 0)).reshape(d_dim, nt)
    return {
        "xdt": xdt.astype(np.float32),
        "ker": np.ascontiguousarray(ker, dtype=np.float32),
        "biasrow": np.ascontiguousarray(bias, dtype=np.float32).reshape(1, U),
        "onesrow": np.ones((1, 512), dtype=np.float32),
        "translhs": np.ascontiguousarray(trans, dtype=np.float32),
        "ident": np.eye(U, dtype=np.float32),
        "lbv": np.ascontiguousarray(lb, dtype=np.float32).reshape(U, 1),
        "rbv": np.ascontiguousarray(rb, dtype=np.float32).reshape(U, 1),
    }


def backtrace(v, trans):
    """v: [b, t, u] forward max values; trans: [u, u]. Returns int tags
    [b, t]."""
    nb, nt, nu = v.shape
    tags = np.zeros((nb, nt), dtype=np.int64)
    cur = np.argmax(v[:, -1, :], axis=1)
    tags[:, -1] = cur
    for t in range(nt - 2, -1, -1):
        scores = v[:, t, :] + trans[:, cur].T     # fp32, same as device order
        cur = np.argmax(scores, axis=1)
        tags[:, t] = cur
    return tags


def vout_to_v(vout_core, t_steps=T):
    """vout [U, (g, t, bb)] -> v [BL, t, U] with b = g * GB + bb."""
    v = vout_core.reshape(U, NG, t_steps, GB)     # [u, g, t, bb]
    return np.ascontiguousarray(v.transpose(1, 3, 2, 0).reshape(BL, t_steps, U))


def kernel(x, kernel, bias, chain_kernel, left_boundary, right_boundary):
    x = np.asarray(x, dtype=np.float32)
    ker = np.asarray(kernel, dtype=np.float32)
    bias = np.asarray(bias, dtype=np.float32)
    trans = np.asarray(chain_kernel, dtype=np.float32)
    lb = np.asarray(left_boundary, dtype=np.float32)
    rb = np.asarray(right_boundary, dtype=np.float32)

    nc = build_program()
    in_maps = [
        make_in_map(x[c * BL:(c + 1) * BL], ker, bias, trans, lb, rb)
        for c in range(NCORES)
    ]
    kwargs = {}
    if os.environ.get("CRF_TRACE"):
        kwargs = {"trace": True, "tmpdir": os.environ.get("CRF_TRACE_DIR") or None}
    res = run_bass_kernel_spmd(nc, in_maps, core_ids=list(range(NCORES)), **kwargs)
    global last_results
    last_results = res
    v = np.concatenate(
        [vout_to_v(np.asarray(r["vout"])) for r in res.results], axis=0)
    tags = backtrace(v, trans)
    return tags.astype(np.float32)
